# revision 34
# baseline (speedup 1.0000x reference)
"""TRN2 Bass kernel: transformer Block (LN->MHA->2x residual->LN->MLP) for
B=32,N=512,C=768,H=12. Data-parallel over batch across 8 NeuronCores (4
items/core). All matmuls in bf16 (fp32 PSUM accumulate); weights are
pre-transposed + pre-converted to bf16 on host and loaded with plain large
contiguous DMAs, so the PE never transposes weights.

Per-core program (single fused pass, no DRAM scratch):
  stage A, per batch item: attention for item b is emitted with the
  PE-dense / ScalarE-light work of item b+1 (LN1, h0 transposes, qk, v)
  interleaved into its (head-pair, kv-chunk) slot loop. This keeps the PE
  busy during the per-slot Exp (ScalarE) dependency and keeps the HAM
  clock-gate warm. Softmax uses no max-subtraction (scores are N(0,1)
  scale); denominators ride the AV matmul as a [v|1] ones column; the
  1/denom broadcast is a pair of row-group-disjoint ones matmuls (they
  overlap on the PE) deferred two slots behind the AV they normalize.
  proj weights are pre-scaled 2x on host (Block's x = 2*attn_out quirk);
  x2 stays resident in SBUF (bf16).
  During the last item's attention, LN2 + transposes of MLP chunk 0 are
  the interleave feed, so stage B starts with fc1 immediately.
  stage B, per 512-token chunk: fc1 -> gelu -> f1 (SBUF) -> fc2 ->
  + x2 + fc2_b -> out, with next-chunk LN2/transposes interleaved.
"""
import numpy as np
import ml_dtypes
from contextlib import ExitStack

import concourse.bass as bass
import concourse.tile as tile
import concourse.bacc as bacc
from concourse import mybir
from concourse.bass_utils import run_bass_kernel_spmd
from concourse.masks import make_identity

F32 = mybir.dt.float32
BF16 = mybir.dt.bfloat16
AF = mybir.ActivationFunctionType
ALU = mybir.AluOpType

B, N, C = 32, 512, 768
H, D = 12, 64
HID = 4 * C
EPS = 1e-5
NCORES = 8
BPC = B // NCORES            # batch items per core
T = BPC * N                  # tokens per core
G = T // 128                 # token tiles per core
CK = C // 128                # 6 contraction chunks over C
JH = HID // 128              # 24 hidden feature tiles
NT = N // 128                # 4 token tiles per item
SCALE = D ** -0.5
PREF = 12                    # fc1 j-tiles pre-fed into last item's attention
BF = ml_dtypes.bfloat16


def _bc(ap, p=128):
    """Broadcast a 1-D DRAM AP across p partitions (stride-0 partition dim)."""
    return bass.AP(tensor=ap.tensor, offset=ap.offset, ap=[[0, p]] + list(ap.ap))


def _emit(tc, io, ctx):
    nc = tc.nc

    consts = ctx.enter_context(tc.tile_pool(name="consts", bufs=1))
    x2pool = ctx.enter_context(tc.tile_pool(name="x2pool", bufs=1))
    wf1p = ctx.enter_context(tc.tile_pool(name="wf1p", bufs=1))
    h2p = ctx.enter_context(tc.tile_pool(name="h2p", bufs=1))
    psw = ctx.enter_context(tc.tile_pool(name="psw", bufs=2, space="PSUM"))
    psf = ctx.enter_context(tc.tile_pool(name="psf", bufs=2, space="PSUM"))
    psav = ctx.enter_context(tc.tile_pool(name="psav", bufs=2, space="PSUM"))
    pstp = ctx.enter_context(tc.tile_pool(name="pstp", bufs=2, space="PSUM"))

    # ---------------- constants ----------------
    ident32 = consts.tile([128, 128], F32)
    make_identity(nc, ident32)
    identb = consts.tile([128, 128], BF16)
    nc.vector.tensor_copy(out=identb, in_=ident32)
    # ones rows for the 1/denom broadcast matmuls (lhsT = ones64[r:r+1, :])
    ones64 = consts.tile([128, 64], BF16)
    nc.vector.memset(ones64, 1.0)
    epst = consts.tile([128, 1], F32)
    nc.vector.memset(epst, EPS)

    # per-channel LN params in transposed-chunk layout: [p, k] = w[128k+p]
    ln1w_k = consts.tile([128, CK], F32)
    nc.scalar.dma_start(out=ln1w_k, in_=io["ln1_w"].rearrange("(k p) -> p k", p=128))
    ln1b_k = consts.tile([128, CK], F32)
    nc.scalar.dma_start(out=ln1b_k, in_=io["ln1_b"].rearrange("(k p) -> p k", p=128))
    ln2w_k = consts.tile([128, CK], F32)
    nc.scalar.dma_start(out=ln2w_k, in_=io["ln2_w"].rearrange("(k p) -> p k", p=128))
    ln2b_k = consts.tile([128, CK], F32)
    nc.scalar.dma_start(out=ln2b_k, in_=io["ln2_b"].rearrange("(k p) -> p k", p=128))
    pb2_bc = consts.tile([128, C], F32)
    nc.scalar.dma_start(out=pb2_bc, in_=_bc(io["pb2"]))
    fc2b_bc = consts.tile([128, C], F32)
    nc.scalar.dma_start(out=fc2b_bc, in_=_bc(io["fc2_b"]))
    fc1b_t = consts.tile([128, JH], F32)
    nc.scalar.dma_start(out=fc1b_t, in_=io["fc1_b"].rearrange("(j p) -> p j", p=128))

    # x2 residual stream, resident bf16 [128, G, C]
    x2r = x2pool.tile([128, G, C], BF16)
    # fc1 weights (DMA overlaps stage A compute)
    wf1T = wf1p.tile([128, CK, HID], BF16)

    def load_wT(wT_ap, nrows, ncols, dst):
        """wT [ncols, nrows] DRAM bf16 (host-pre-transposed) ->
        dst [128, ncols//128, nrows]; dst[p, k, r] = wT[128k+p, r]."""
        for k in range(ncols // 128):
            nc.sync.dma_start(
                out=dst[:, k, :], in_=wT_ap[k * 128:(k + 1) * 128, :])

    def layer_norm(x_t, pool):
        """x_t [128, C] bf16 -> xn [128, C] bf16 = (x - mu) * rstd."""
        st = pool.tile([128, 3, nc.vector.BN_STATS_DIM], F32, tag="bnst",
                       bufs=3, name="st")
        for i in range(3):
            nc.vector.bn_stats(out=st[:, i, :], in_=x_t[:, 256 * i:256 * (i + 1)])
        mv = pool.tile([128, nc.vector.BN_AGGR_DIM], F32, tag="mv", bufs=3,
                       name="mv")
        nc.vector.bn_aggr(out=mv, in_=st)
        rstd = pool.tile([128, 1], F32, tag="rstd", bufs=3, name="rstd")
        nc.scalar.activation(out=rstd, in_=mv[:, 1:2], func=AF.Sqrt, bias=epst)
        nc.vector.reciprocal(out=rstd, in_=rstd)
        xn = pool.tile([128, C], BF16, tag="xn", bufs=3, name="xn")
        nc.vector.tensor_scalar(out=xn, in0=x_t, scalar1=mv[:, 0:1],
                                scalar2=rstd, op0=ALU.subtract, op1=ALU.mult)
        return xn

    def transpose_block(xn, dstT, tt, w_k, b_k):
        """xn [128, C] bf16 -> dstT[:, k, tt*128:(tt+1)*128] = xn.T * w + b."""
        for k in range(CK):
            tp = pstp.tile([128, 128], BF16, tag="tp", name="tp",
                           padded_shape=[128, 1024])
            nc.tensor.transpose(tp[:], xn[:, k * 128:(k + 1) * 128], identb[:])
            nc.vector.tensor_scalar(
                out=dstT[:, k, tt * 128:(tt + 1) * 128], in0=tp[:],
                scalar1=w_k[:, k:k + 1], scalar2=b_k[:, k:k + 1],
                op0=ALU.mult, op1=ALU.add)

    # ================= stage A =================
    with tc.tile_pool(name="wqkvp", bufs=1) as wqkvp, \
         tc.tile_pool(name="wpp", bufs=1) as wpp, \
         tc.tile_pool(name="p1", bufs=1) as p1, \
         tc.tile_pool(name="xio", bufs=1) as xio:

        wqkvT = wqkvp.tile([128, CK, 3 * C], BF16)
        load_wT(io["qkv_wT"], 3 * C, C, wqkvT)
        wpT = wpp.tile([128, CK, C], BF16)
        load_wT(io["proj_wT"], C, C, wpT)
        # prefetch fc1 weights; DMA executes during stage A compute
        load_wT(io["fc1_wT"], HID, C, wf1T)

        def load_x(b):
            xts = []
            for tt in range(NT):
                t0 = b * N + tt * 128
                x_t = xio.tile([128, C], BF16, tag="xio", bufs=4, name="x_t")
                nc.scalar.dma_start(out=x_t, in_=io["x"][t0:t0 + 128, :])
                xts.append(x_t)
            return xts

        def emit_qk(j, qk_sb, h0T):
            qp = psf.tile([128, N], F32, tag="f", name="qp")
            for k in range(CK):
                nc.tensor.matmul(qp[:], wqkvT[:, k, j * 128:(j + 1) * 128],
                                 h0T[:, k, :], start=(k == 0),
                                 stop=(k == CK - 1))
            # evac on DVE: ScalarE is exp-saturated while feed groups run
            nc.vector.tensor_copy(out=qk_sb[:, j, :], in_=qp[:])

        def emit_vw(tt, v_sb, h0T):
            vw = psf.tile([128, 512], F32, tag="f", name="vw")
            for k in range(CK):
                nc.tensor.matmul(vw[:], h0T[:, k, tt * 128:(tt + 1) * 128],
                                 wqkvT[:, k, 2 * C:2 * C + 512],
                                 start=(k == 0), stop=(k == CK - 1))
            nc.vector.tensor_copy(out=v_sb[:, tt, 0:8, 0:D],
                                  in_=vw.rearrange("p (h d) -> p h d", h=8))

        def emit_vh(tt, v_sb, h0T):
            vh = psf.tile([128, 512], F32, tag="f", name="vh")
            for k in range(CK):
                nc.tensor.matmul(vh[:, 0:256],
                                 h0T[:, k, tt * 128:(tt + 1) * 128],
                                 wqkvT[:, k, 2 * C + 512:3 * C],
                                 start=(k == 0), stop=(k == CK - 1))
            nc.vector.tensor_copy(
                out=v_sb[:, tt, 8:12, 0:D],
                in_=vh[:, 0:256].rearrange("p (h d) -> p h d", h=4))

        def item_state(b):
            """Allocate next item's tiles + the feed groups producing them."""
            xts = load_x(b)
            st = {
                "h0T": p1.tile([128, CK, N], BF16, tag="h0T", bufs=2,
                               name="h0T"),
                "qk": p1.tile([128, 2 * CK, N], BF16, tag="qk", bufs=2,
                              name="qk_sb"),
                "v": p1.tile([128, NT, H, D + 1], BF16, tag="v", bufs=2,
                             name="v_sb"),
                "xn": [None] * NT,
            }
            nc.gpsimd.memset(st["v"][:, :, :, D:D + 1], 1.0)
            feed = []

            def ln_t(tt):
                st["xn"][tt] = layer_norm(xts[tt], p1)
                transpose_block(st["xn"][tt], st["h0T"], tt, ln1w_k, ln1b_k)

            for tt in range(NT):
                feed.append(lambda tt=tt: ln_t(tt))
            for j in range(2 * CK):
                feed.append(lambda j=j: emit_qk(j, st["qk"], st["h0T"]))
            for tt in range(NT):
                feed.append(lambda tt=tt: emit_vw(tt, st["v"], st["h0T"]))
                feed.append(lambda tt=tt: emit_vh(tt, st["v"], st["h0T"]))
            return st, feed

        def mlp0_feed():
            """Feed for the last item: LN2 + transposes + the first PREF
            fc1 j-tiles of MLP chunk 0, so its attention stays PE-dense."""
            st = {"xn": [None] * NT}
            h2T = h2p.tile([128, CK, N], BF16, tag="h2T", bufs=2, name="h2T0")
            f1 = h2p.tile([128, PREF, N], BF16, tag="f1a", bufs=1,
                          name="f1a_0")
            feed = []

            def ln2_t(tt):
                st["xn"][tt] = layer_norm(x2r[:, tt, :], p1)
                transpose_block(st["xn"][tt], h2T, tt, ln2w_k, ln2b_k)

            for tt in range(NT):
                feed.append(lambda tt=tt: ln2_t(tt))

            def fc1_j(j):
                fp = psf.tile([128, N], F32, tag="f", name="fp")
                for k in range(CK):
                    nc.tensor.matmul(fp[:], wf1T[:, k, j * 128:(j + 1) * 128],
                                     h2T[:, k, :], start=(k == 0),
                                     stop=(k == CK - 1))
                nc.scalar.activation(out=f1[:, j, :], in_=fp[:], func=AF.Gelu,
                                     bias=fc1b_t[:, j:j + 1])

            for j in range(PREF):
                feed.append(lambda j=j: fc1_j(j))
            return h2T, f1, feed

        # prologue: item 0 produced un-interleaved
        cur, feed0 = item_state(0)
        for fn in feed0:
            fn()

        h2T0 = f1_0 = None
        for b in range(BPC):
            if b + 1 < BPC:
                nxt, feed = item_state(b + 1)
            else:
                h2T0, f1_0, feed = mlp0_feed()
            qk_sb, v_sb = cur["qk"], cur["v"]

            oT = p1.tile([128, CK, N], BF16, tag="oT", bufs=1, name="oT")
            slots = [(hp, c) for hp in range(CK) for c in range(NT)]
            ex_sb = {}
            av_ps = {}
            done = {}
            fi = [0]

            def feed_step():
                if fi[0] < len(feed):
                    feed[fi[0]]()
                    fi[0] += 1

            def emit_sc(hp, c):
                scp = psw.tile([128, N], F32, tag="w", name="scp")
                scq = psw.tile([128, N], F32, tag="w", name="scq")
                kj = CK + hp
                nc.tensor.matmul(scp[:],
                                 qk_sb[0:64, kj, c * 128:(c + 1) * 128],
                                 qk_sb[0:64, hp, :])
                nc.tensor.matmul(scq[:],
                                 qk_sb[64:128, kj, c * 128:(c + 1) * 128],
                                 qk_sb[64:128, hp, :])
                exa = p1.tile([128, N], BF16, tag="ex", bufs=5, name="exa")
                nc.scalar.activation(out=exa, in_=scp[:], func=AF.Exp,
                                     scale=SCALE)
                exb = p1.tile([128, N], BF16, tag="ex", bufs=5, name="exb")
                nc.scalar.activation(out=exb, in_=scq[:], func=AF.Exp,
                                     scale=SCALE)
                ex_sb[(hp, c)] = (exa, exb)

            def emit_av(hp, c):
                if c == 0:
                    av_ps[hp] = (
                        psav.tile([128, N], F32, tag="av", name="ava"),
                        psav.tile([128, N], F32, tag="av", name="avb"))
                ava, avb = av_ps[hp]
                exa, exb = ex_sb.pop((hp, c))
                nc.tensor.matmul(ava[0:D + 1, :], v_sb[:, c, 2 * hp, :],
                                 exa[:], start=(c == 0), stop=(c == NT - 1))
                nc.tensor.matmul(avb[0:D + 1, :], v_sb[:, c, 2 * hp + 1, :],
                                 exb[:], start=(c == 0), stop=(c == NT - 1))

            def finish_pair(hp):
                """Spill av pair to SBUF (heads at partitions 0:64/64:128),
                gather denom rows at partitions 0/32, 1/x to bf16."""
                ava, avb = av_ps.pop(hp)
                sr = p1.tile([128, N], F32, tag="srow", bufs=2, name="sr")
                nc.vector.tensor_copy(out=sr[0:1, :], in_=ava[D:D + 1, :])
                nc.scalar.copy(out=sr[32:33, :], in_=avb[D:D + 1, :])
                avs = p1.tile([128, N], BF16, tag="avs", bufs=3, name="avs")
                nc.vector.tensor_copy(out=avs[0:D, :], in_=ava[0:D, :])
                nc.vector.tensor_copy(out=avs[64:128, :], in_=avb[0:D, :])
                rc = p1.tile([128, N], F32, tag="srow", bufs=2, name="rc")
                nc.vector.reciprocal_approx_fast(out=rc[0:33, :],
                                                 in_=sr[0:33, :])
                rcb = p1.tile([128, N], BF16, tag="rcb", bufs=2, name="rcb")
                nc.vector.tensor_copy(out=rcb[0:33, :], in_=rc[0:33, :])
                done[hp] = (avs, rcb)

            def emit_bcast(hp):
                """Two row-group-disjoint broadcast matmuls (run
                concurrently on the PE) + two normalize muls into oT."""
                avs, rcb = done.pop(hp)
                for sub in range(2):
                    r = 32 * sub
                    bcp = psf.tile([128, N], F32, tag="f", name="bcp")
                    nc.tensor.matmul(bcp[0:64, :], ones64[r:r + 1, :],
                                     rcb[r:r + 1, :], tile_position=(r, 0))
                    nc.vector.tensor_mul(
                        out=oT[64 * sub:64 * (sub + 1), hp, :],
                        in0=avs[64 * sub:64 * (sub + 1), :],
                        in1=bcp[0:64, :])

            for i, (hp, c) in enumerate(slots):
                emit_sc(hp, c)
                feed_step()
                emit_av(hp, c)
                if c == NT - 1:
                    finish_pair(hp)
                if c == 1 and hp > 0:
                    emit_bcast(hp - 1)
            emit_bcast(CK - 1)

            # ---- proj (+2x via pre-scaled weights) ----
            for tt in range(NT):
                pw = psf.tile([128, 512], F32, tag="f", name="pw")
                ph = psf.tile([128, 512], F32, tag="f", name="ph")
                for k in range(CK):
                    nc.tensor.matmul(pw[:], oT[:, k, tt * 128:(tt + 1) * 128],
                                     wpT[:, k, 0:512],
                                     start=(k == 0), stop=(k == CK - 1))
                    nc.tensor.matmul(ph[:, 0:256],
                                     oT[:, k, tt * 128:(tt + 1) * 128],
                                     wpT[:, k, 512:768],
                                     start=(k == 0), stop=(k == CK - 1))
                feed_step()
                g = b * NT + tt
                nc.vector.tensor_add(out=x2r[:, g, 0:512], in0=pw[:],
                                     in1=pb2_bc[:, 0:512])
                nc.vector.tensor_add(out=x2r[:, g, 512:768], in0=ph[:, 0:256],
                                     in1=pb2_bc[:, 512:768])
            while fi[0] < len(feed):
                feed_step()
            if b + 1 < BPC:
                cur = nxt

    # ================= stage B: MLP =================
    with tc.tile_pool(name="wf2p", bufs=1) as wf2p, \
         tc.tile_pool(name="p2", bufs=1) as p2:
        wf2T = wf2p.tile([128, JH, C], BF16)
        load_wT(io["fc2_wT"], C, HID, wf2T)

        def ln2_chunk(ch):
            return [layer_norm(x2r[:, ch * NT + tt, :], p2)
                    for tt in range(NT)]

        h2T_cur, f1_0p = h2T0, f1_0
        for ch in range(G // NT):
            # ---- fc1 + gelu ----
            f1a = (f1_0p if ch == 0 else
                   h2p.tile([128, PREF, N], BF16, tag="f1a", bufs=1,
                            name="f1a"))
            f1b = p2.tile([128, JH - PREF, N], BF16, tag="f1b", bufs=1,
                          name="f1b")

            def f1_at(j):
                return f1a[:, j, :] if j < PREF else f1b[:, j - PREF, :]

            for j in range(PREF if ch == 0 else 0, JH):
                fp = psw.tile([128, N], F32, tag="w", name="fp")
                for k in range(CK):
                    nc.tensor.matmul(fp[:], wf1T[:, k, j * 128:(j + 1) * 128],
                                     h2T_cur[:, k, :], start=(k == 0),
                                     stop=(k == CK - 1))
                nc.scalar.activation(out=f1_at(j), in_=fp[:], func=AF.Gelu,
                                     bias=fc1b_t[:, j:j + 1])

            # LN2 of next chunk on DVE while fc1 runs
            if ch + 1 < G // NT:
                xns_n = ln2_chunk(ch + 1)
                h2T_next = h2p.tile([128, CK, N], BF16, tag="h2T", bufs=2,
                                    name="h2Tn")

            # ---- fc2 + residual, interleaved with next chunk transposes ----
            for tt in range(NT):
                g = ch * NT + tt
                x2pb = p2.tile([128, C], F32, tag="x2pb", bufs=2, name="x2pb")
                nc.vector.tensor_add(out=x2pb, in0=x2r[:, g, :], in1=fc2b_bc)
                f2a = psf.tile([128, 512], F32, tag="f", name="f2a")
                f2b = psf.tile([128, 512], F32, tag="f", name="f2b")
                for k in range(JH):
                    lhs = f1_at(k)[:, tt * 128:(tt + 1) * 128]
                    nc.tensor.matmul(f2a[:], lhs, wf2T[:, k, 0:512],
                                     start=(k == 0), stop=(k == JH - 1))
                    nc.tensor.matmul(f2b[:, 0:256], lhs, wf2T[:, k, 512:768],
                                     start=(k == 0), stop=(k == JH - 1))
                if ch + 1 < G // NT:
                    transpose_block(xns_n[tt], h2T_next, tt, ln2w_k, ln2b_k)
                o_t = p2.tile([128, C], F32, tag="outt", bufs=3, name="o_t")
                nc.vector.tensor_add(out=o_t[:, 0:512], in0=f2a[:],
                                     in1=x2pb[:, 0:512])
                nc.vector.tensor_add(out=o_t[:, 512:768], in0=f2b[:, 0:256],
                                     in1=x2pb[:, 512:768])
                nc.scalar.dma_start(
                    out=io["out"][g * 128:(g + 1) * 128, :], in_=o_t)
            if ch + 1 < G // NT:
                h2T_cur = h2T_next


_CACHE = {}


def _build():
    if "nc" in _CACHE:
        return _CACHE["nc"]
    nc = bacc.Bacc("TRN2", target_bir_lowering=False, debug=False,
                   num_devices=NCORES)
    io = {}
    io["x"] = nc.dram_tensor("x", [T, C], BF16, kind="ExternalInput").ap()
    for name, shape in [("qkv_wT", [C, 3 * C]), ("proj_wT", [C, C]),
                        ("fc1_wT", [C, HID]), ("fc2_wT", [HID, C])]:
        io[name] = nc.dram_tensor(name, shape, BF16, kind="ExternalInput").ap()
    for name, shape in [("ln1_w", [C]), ("ln1_b", [C]), ("ln2_w", [C]),
                        ("ln2_b", [C]), ("pb2", [C]), ("fc1_b", [HID]),
                        ("fc2_b", [C])]:
        io[name] = nc.dram_tensor(name, shape, F32, kind="ExternalInput").ap()
    io["out"] = nc.dram_tensor("out", [T, C], F32, kind="ExternalOutput").ap()

    with tile.TileContext(nc) as tc:
        with ExitStack() as ctx:
            _emit(tc, io, ctx)
    nc.compile()
    _CACHE["nc"] = nc
    return nc


def _in_maps(inputs):
    f = {k: np.ascontiguousarray(np.asarray(v, dtype=np.float32))
         for k, v in inputs.items()}
    x = f["x"].reshape(B, N, C)
    base = {
        "qkv_wT": np.ascontiguousarray(f["qkv_w"].T).astype(BF),
        "proj_wT": np.ascontiguousarray(2.0 * f["proj_w"].T).astype(BF),
        "fc1_wT": np.ascontiguousarray(f["fc1_w"].T).astype(BF),
        "fc2_wT": np.ascontiguousarray(f["fc2_w"].T).astype(BF),
        "ln1_w": f["ln1_w"], "ln1_b": f["ln1_b"],
        "ln2_w": f["ln2_w"], "ln2_b": f["ln2_b"],
        "pb2": 2.0 * f["proj_b"], "fc1_b": f["fc1_b"], "fc2_b": f["fc2_b"],
    }
    in_maps = []
    for c in range(NCORES):
        m = dict(base)
        m["x"] = np.ascontiguousarray(
            x[c * BPC:(c + 1) * BPC].reshape(T, C).astype(BF))
        in_maps.append(m)
    return in_maps


def kernel(**inputs):
    nc = _build()
    in_maps = _in_maps(inputs)
    res = run_bass_kernel_spmd(nc, in_maps, core_ids=list(range(NCORES)))
    out = np.concatenate(
        [r["out"].reshape(BPC, N, C) for r in res.results], axis=0)
    return out.astype(np.float32)


if __name__ == "__main__":
    rng = np.random.default_rng(0)
    ins = {
        "x": rng.standard_normal((B, N, C), dtype=np.float32),
        "ln1_w": np.ones(C, np.float32), "ln1_b": np.zeros(C, np.float32),
        "qkv_w": rng.standard_normal((3 * C, C), dtype=np.float32) / np.sqrt(C),
        "proj_w": rng.standard_normal((C, C), dtype=np.float32) / np.sqrt(C),
        "proj_b": np.zeros(C, np.float32),
        "ln2_w": np.ones(C, np.float32), "ln2_b": np.zeros(C, np.float32),
        "fc1_w": rng.standard_normal((HID, C), dtype=np.float32) / np.sqrt(C),
        "fc1_b": np.zeros(HID, np.float32),
        "fc2_w": rng.standard_normal((C, HID), dtype=np.float32) / np.sqrt(HID),
        "fc2_b": np.zeros(C, np.float32),
    }
    out = kernel(**ins)

    def ln(x, w, b):
        mu = x.mean(-1, keepdims=True)
        va = x.var(-1, keepdims=True)
        return (x - mu) / np.sqrt(va + EPS) * w + b

    x = ins["x"]
    h = ln(x, ins["ln1_w"], ins["ln1_b"])
    qkv = np.einsum('bnc,fc->bnf', h, ins["qkv_w"])
    qkv = qkv.reshape(B, N, 3, H, D).transpose(2, 0, 3, 1, 4)
    q, k, v = qkv[0], qkv[1], qkv[2]
    att = np.einsum('bhnd,bhmd->bhnm', q, k) * SCALE
    att = np.exp(att - att.max(-1, keepdims=True))
    att = att / att.sum(-1, keepdims=True)
    o = np.einsum('bhnm,bhmd->bhnd', att, v)
    o = o.transpose(0, 2, 1, 3).reshape(B, N, C)
    o = np.einsum('bnc,oc->bno', o, ins["proj_w"]) + ins["proj_b"]
    x2 = 2 * o
    h2 = ln(x2, ins["ln2_w"], ins["ln2_b"])
    h2 = np.einsum('bnc,hc->bnh', h2, ins["fc1_w"]) + ins["fc1_b"]
    from scipy.special import erf
    h2 = h2 * 0.5 * (1 + erf(h2 / np.sqrt(2)))
    h2 = np.einsum('bnh,oh->bno', h2, ins["fc2_w"]) + ins["fc2_b"]
    ref = x2 + h2
    err = np.abs(out - ref)
    print("out", out.shape, "absmax", np.abs(ref).max(),
          "maxerr", err.max(), "rel", err.max() / np.abs(ref).max())


# revision 36
# speedup vs baseline: 1.0057x; 1.0057x over previous
"""TRN2 Bass kernel: transformer Block (LN->MHA->2x residual->LN->MLP) for
B=32,N=512,C=768,H=12. Data-parallel over batch across 8 NeuronCores (4
items/core). All matmuls in bf16 (fp32 PSUM accumulate); weights are
pre-transposed + pre-converted to bf16 on host and loaded with plain large
contiguous DMAs, so the PE never transposes weights.

Per-core program (single fused pass, no DRAM scratch):
  stage A, per batch item: attention for item b is emitted with the
  PE-dense / ScalarE-light work of item b+1 (LN1, h0 transposes, qk, v)
  interleaved into its (head-pair, kv-chunk) slot loop. This keeps the PE
  busy during the per-slot Exp (ScalarE) dependency and keeps the HAM
  clock-gate warm. Softmax uses no max-subtraction (scores are N(0,1)
  scale); denominators ride the AV matmul as a [v|1] ones column; the
  1/denom broadcast is a pair of row-group-disjoint ones matmuls (they
  overlap on the PE) deferred two slots behind the AV they normalize.
  proj weights are pre-scaled 2x on host (Block's x = 2*attn_out quirk);
  x2 stays resident in SBUF (bf16).
  During the last item's attention, LN2 + transposes of MLP chunk 0 are
  the interleave feed, so stage B starts with fc1 immediately.
  stage B, per 512-token chunk: fc1 -> gelu -> f1 (SBUF) -> fc2 ->
  + x2 + fc2_b -> out, with next-chunk LN2/transposes interleaved.
"""
import numpy as np
import ml_dtypes
from contextlib import ExitStack

import concourse.bass as bass
import concourse.tile as tile
import concourse.bacc as bacc
from concourse import mybir
from concourse.bass_utils import run_bass_kernel_spmd
from concourse.masks import make_identity

F32 = mybir.dt.float32
BF16 = mybir.dt.bfloat16
AF = mybir.ActivationFunctionType
ALU = mybir.AluOpType

B, N, C = 32, 512, 768
H, D = 12, 64
HID = 4 * C
EPS = 1e-5
NCORES = 8
BPC = B // NCORES            # batch items per core
T = BPC * N                  # tokens per core
G = T // 128                 # token tiles per core
CK = C // 128                # 6 contraction chunks over C
JH = HID // 128              # 24 hidden feature tiles
NT = N // 128                # 4 token tiles per item
SCALE = D ** -0.5
PREF = 12                    # fc1 j-tiles pre-fed into last item's attention
BF = ml_dtypes.bfloat16


def _bc(ap, p=128):
    """Broadcast a 1-D DRAM AP across p partitions (stride-0 partition dim)."""
    return bass.AP(tensor=ap.tensor, offset=ap.offset, ap=[[0, p]] + list(ap.ap))


def _emit(tc, io, ctx):
    nc = tc.nc

    consts = ctx.enter_context(tc.tile_pool(name="consts", bufs=1))
    x2pool = ctx.enter_context(tc.tile_pool(name="x2pool", bufs=1))
    wf1p = ctx.enter_context(tc.tile_pool(name="wf1p", bufs=1))
    h2p = ctx.enter_context(tc.tile_pool(name="h2p", bufs=1))
    psw = ctx.enter_context(tc.tile_pool(name="psw", bufs=2, space="PSUM"))
    psf = ctx.enter_context(tc.tile_pool(name="psf", bufs=2, space="PSUM"))
    psav = ctx.enter_context(tc.tile_pool(name="psav", bufs=2, space="PSUM"))
    pstp = ctx.enter_context(tc.tile_pool(name="pstp", bufs=2, space="PSUM"))

    # ---------------- constants ----------------
    ident32 = consts.tile([128, 128], F32)
    make_identity(nc, ident32)
    identb = consts.tile([128, 128], BF16)
    nc.vector.tensor_copy(out=identb, in_=ident32)
    # ones rows for the 1/denom broadcast matmuls (lhsT = ones64[r:r+1, :])
    ones64 = consts.tile([128, 64], BF16)
    nc.vector.memset(ones64, 1.0)
    epst = consts.tile([128, 1], F32)
    nc.vector.memset(epst, EPS)

    # per-channel LN params in transposed-chunk layout: [p, k] = w[128k+p]
    ln1w_k = consts.tile([128, CK], F32)
    nc.scalar.dma_start(out=ln1w_k, in_=io["ln1_w"].rearrange("(k p) -> p k", p=128))
    ln1b_k = consts.tile([128, CK], F32)
    nc.scalar.dma_start(out=ln1b_k, in_=io["ln1_b"].rearrange("(k p) -> p k", p=128))
    ln2w_k = consts.tile([128, CK], F32)
    nc.scalar.dma_start(out=ln2w_k, in_=io["ln2_w"].rearrange("(k p) -> p k", p=128))
    ln2b_k = consts.tile([128, CK], F32)
    nc.scalar.dma_start(out=ln2b_k, in_=io["ln2_b"].rearrange("(k p) -> p k", p=128))
    pb2_bc = consts.tile([128, C], F32)
    nc.scalar.dma_start(out=pb2_bc, in_=_bc(io["pb2"]))
    fc2b_bc = consts.tile([128, C], F32)
    nc.scalar.dma_start(out=fc2b_bc, in_=_bc(io["fc2_b"]))
    fc1b_t = consts.tile([128, JH], F32)
    nc.scalar.dma_start(out=fc1b_t, in_=io["fc1_b"].rearrange("(j p) -> p j", p=128))

    # x2 residual stream, resident bf16 [128, G, C]
    x2r = x2pool.tile([128, G, C], BF16)
    # fc1 weights (DMA overlaps stage A compute)
    wf1T = wf1p.tile([128, CK, HID], BF16)

    def load_wT(wT_ap, nrows, ncols, dst):
        """wT [ncols, nrows] DRAM bf16 (host-pre-transposed) ->
        dst [128, ncols//128, nrows]; dst[p, k, r] = wT[128k+p, r]."""
        for k in range(ncols // 128):
            nc.sync.dma_start(
                out=dst[:, k, :], in_=wT_ap[k * 128:(k + 1) * 128, :])

    def layer_norm(x_t, pool):
        """x_t [128, C] bf16 -> xn [128, C] bf16 = (x - mu) * rstd."""
        st = pool.tile([128, 3, nc.vector.BN_STATS_DIM], F32, tag="bnst",
                       bufs=3, name="st")
        for i in range(3):
            nc.vector.bn_stats(out=st[:, i, :], in_=x_t[:, 256 * i:256 * (i + 1)])
        mv = pool.tile([128, nc.vector.BN_AGGR_DIM], F32, tag="mv", bufs=3,
                       name="mv")
        nc.vector.bn_aggr(out=mv, in_=st)
        rstd = pool.tile([128, 1], F32, tag="rstd", bufs=3, name="rstd")
        nc.scalar.activation(out=rstd, in_=mv[:, 1:2], func=AF.Sqrt, bias=epst)
        nc.vector.reciprocal(out=rstd, in_=rstd)
        xn = pool.tile([128, C], BF16, tag="xn", bufs=3, name="xn")
        nc.vector.tensor_scalar(out=xn, in0=x_t, scalar1=mv[:, 0:1],
                                scalar2=rstd, op0=ALU.subtract, op1=ALU.mult)
        return xn

    def transpose_block(xn, dstT, tt, w_k, b_k):
        """xn [128, C] bf16 -> dstT[:, k, tt*128:(tt+1)*128] = xn.T * w + b."""
        for k in range(CK):
            tp = pstp.tile([128, 128], BF16, tag="tp", name="tp",
                           padded_shape=[128, 1024])
            nc.tensor.transpose(tp[:], xn[:, k * 128:(k + 1) * 128], identb[:])
            nc.vector.tensor_scalar(
                out=dstT[:, k, tt * 128:(tt + 1) * 128], in0=tp[:],
                scalar1=w_k[:, k:k + 1], scalar2=b_k[:, k:k + 1],
                op0=ALU.mult, op1=ALU.add)

    # ================= stage A =================
    with tc.tile_pool(name="wqkvp", bufs=1) as wqkvp, \
         tc.tile_pool(name="wpp", bufs=1) as wpp, \
         tc.tile_pool(name="p1", bufs=1) as p1, \
         tc.tile_pool(name="xio", bufs=1) as xio:

        wqkvT = wqkvp.tile([128, CK, 3 * C], BF16)
        load_wT(io["qkv_wT"], 3 * C, C, wqkvT)
        wpT = wpp.tile([128, CK, C], BF16)
        load_wT(io["proj_wT"], C, C, wpT)
        # prefetch fc1 weights; DMA executes during stage A compute
        load_wT(io["fc1_wT"], HID, C, wf1T)

        def load_x(b):
            xts = []
            for tt in range(NT):
                t0 = b * N + tt * 128
                x_t = xio.tile([128, C], BF16, tag="xio", bufs=4, name="x_t")
                nc.scalar.dma_start(out=x_t, in_=io["x"][t0:t0 + 128, :])
                xts.append(x_t)
            return xts

        def emit_qk(j, qk_sb, h0T):
            qp = psf.tile([128, N], F32, tag="f", name="qp")
            for k in range(CK):
                nc.tensor.matmul(qp[:], wqkvT[:, k, j * 128:(j + 1) * 128],
                                 h0T[:, k, :], start=(k == 0),
                                 stop=(k == CK - 1))
            # evac on DVE: ScalarE is exp-saturated while feed groups run
            nc.vector.tensor_copy(out=qk_sb[:, j, :], in_=qp[:])

        def emit_v(tt, v_sb, h0T):
            vw = psf.tile([128, 512], F32, tag="f", name="vw")
            vh = psf.tile([128, 512], F32, tag="f", name="vh")
            for k in range(CK):
                nc.tensor.matmul(vw[:], h0T[:, k, tt * 128:(tt + 1) * 128],
                                 wqkvT[:, k, 2 * C:2 * C + 512],
                                 start=(k == 0), stop=(k == CK - 1))
                nc.tensor.matmul(vh[:, 0:256],
                                 h0T[:, k, tt * 128:(tt + 1) * 128],
                                 wqkvT[:, k, 2 * C + 512:3 * C],
                                 start=(k == 0), stop=(k == CK - 1))
            nc.vector.tensor_copy(out=v_sb[:, tt, 0:8, 0:D],
                                  in_=vw.rearrange("p (h d) -> p h d", h=8))
            nc.vector.tensor_copy(
                out=v_sb[:, tt, 8:12, 0:D],
                in_=vh[:, 0:256].rearrange("p (h d) -> p h d", h=4))

        def item_state(b):
            """Allocate next item's tiles + the feed groups producing them."""
            xts = load_x(b)
            st = {
                "h0T": p1.tile([128, CK, N], BF16, tag="h0T", bufs=2,
                               name="h0T"),
                "qk": p1.tile([128, 2 * CK, N], BF16, tag="qk", bufs=2,
                              name="qk_sb"),
                "v": p1.tile([128, NT, H, D + 1], BF16, tag="v", bufs=2,
                             name="v_sb"),
                "xn": [None] * NT,
            }
            nc.gpsimd.memset(st["v"][:, :, :, D:D + 1], 1.0)
            feed = []

            def ln_t(tt):
                st["xn"][tt] = layer_norm(xts[tt], p1)
                transpose_block(st["xn"][tt], st["h0T"], tt, ln1w_k, ln1b_k)

            for tt in range(NT):
                feed.append(lambda tt=tt: ln_t(tt))
            for j in range(2 * CK):
                feed.append(lambda j=j: emit_qk(j, st["qk"], st["h0T"]))
            for tt in range(NT):
                feed.append(lambda tt=tt: emit_v(tt, st["v"], st["h0T"]))
            return st, feed

        def mlp0_feed():
            """Feed for the last item: LN2 + transposes + the first PREF
            fc1 j-tiles of MLP chunk 0, so its attention stays PE-dense."""
            st = {"xn": [None] * NT}
            h2T = h2p.tile([128, CK, N], BF16, tag="h2T", bufs=2, name="h2T0")
            f1 = h2p.tile([128, PREF, N], BF16, tag="f1a", bufs=1,
                          name="f1a_0")
            feed = []

            def ln2_t(tt):
                st["xn"][tt] = layer_norm(x2r[:, tt, :], p1)
                transpose_block(st["xn"][tt], h2T, tt, ln2w_k, ln2b_k)

            for tt in range(NT):
                feed.append(lambda tt=tt: ln2_t(tt))

            def fc1_j(j):
                fp = psf.tile([128, N], F32, tag="f", name="fp")
                for k in range(CK):
                    nc.tensor.matmul(fp[:], wf1T[:, k, j * 128:(j + 1) * 128],
                                     h2T[:, k, :], start=(k == 0),
                                     stop=(k == CK - 1))
                nc.scalar.activation(out=f1[:, j, :], in_=fp[:], func=AF.Gelu,
                                     bias=fc1b_t[:, j:j + 1])

            for j in range(PREF):
                feed.append(lambda j=j: fc1_j(j))
            return h2T, f1, feed

        # prologue: item 0 produced un-interleaved
        cur, feed0 = item_state(0)
        for fn in feed0:
            fn()

        h2T0 = f1_0 = None
        for b in range(BPC):
            if b + 1 < BPC:
                nxt, feed = item_state(b + 1)
            else:
                h2T0, f1_0, feed = mlp0_feed()
            qk_sb, v_sb = cur["qk"], cur["v"]

            oT = p1.tile([128, CK, N], BF16, tag="oT", bufs=1, name="oT")
            slots = [(hp, c) for hp in range(CK) for c in range(NT)]
            ex_sb = {}
            av_ps = {}
            done = {}
            fi = [0]

            def feed_step():
                if fi[0] < len(feed):
                    feed[fi[0]]()
                    fi[0] += 1

            def emit_sc(hp, c):
                scp = psw.tile([128, N], F32, tag="w", name="scp")
                scq = psw.tile([128, N], F32, tag="w", name="scq")
                kj = CK + hp
                nc.tensor.matmul(scp[:],
                                 qk_sb[0:64, kj, c * 128:(c + 1) * 128],
                                 qk_sb[0:64, hp, :])
                nc.tensor.matmul(scq[:],
                                 qk_sb[64:128, kj, c * 128:(c + 1) * 128],
                                 qk_sb[64:128, hp, :])
                exa = p1.tile([128, N], BF16, tag="ex", bufs=5, name="exa")
                nc.scalar.activation(out=exa, in_=scp[:], func=AF.Exp,
                                     scale=SCALE)
                exb = p1.tile([128, N], BF16, tag="ex", bufs=5, name="exb")
                nc.scalar.activation(out=exb, in_=scq[:], func=AF.Exp,
                                     scale=SCALE)
                ex_sb[(hp, c)] = (exa, exb)

            def emit_av(hp, c):
                if c == 0:
                    av_ps[hp] = (
                        psav.tile([128, N], F32, tag="av", name="ava"),
                        psav.tile([128, N], F32, tag="av", name="avb"))
                ava, avb = av_ps[hp]
                exa, exb = ex_sb.pop((hp, c))
                nc.tensor.matmul(ava[0:D + 1, :], v_sb[:, c, 2 * hp, :],
                                 exa[:], start=(c == 0), stop=(c == NT - 1))
                nc.tensor.matmul(avb[0:D + 1, :], v_sb[:, c, 2 * hp + 1, :],
                                 exb[:], start=(c == 0), stop=(c == NT - 1))

            def finish_pair(hp):
                """Spill av pair to SBUF (heads at partitions 0:64/64:128),
                gather denom rows at partitions 0/32, 1/x to bf16."""
                ava, avb = av_ps.pop(hp)
                sr = p1.tile([128, N], F32, tag="srow", bufs=2, name="sr")
                nc.vector.tensor_copy(out=sr[0:1, :], in_=ava[D:D + 1, :])
                nc.scalar.copy(out=sr[32:33, :], in_=avb[D:D + 1, :])
                avs = p1.tile([128, N], BF16, tag="avs", bufs=3, name="avs")
                nc.vector.tensor_copy(out=avs[0:D, :], in_=ava[0:D, :])
                nc.vector.tensor_copy(out=avs[64:128, :], in_=avb[0:D, :])
                rc = p1.tile([128, N], F32, tag="srow", bufs=2, name="rc")
                nc.vector.reciprocal_approx_fast(out=rc[0:33, :],
                                                 in_=sr[0:33, :])
                rcb = p1.tile([128, N], BF16, tag="rcb", bufs=2, name="rcb")
                nc.vector.tensor_copy(out=rcb[0:33, :], in_=rc[0:33, :])
                done[hp] = (avs, rcb)

            def emit_bcast(hp):
                """Two row-group-disjoint broadcast matmuls (run
                concurrently on the PE) + two normalize muls into oT."""
                avs, rcb = done.pop(hp)
                for sub in range(2):
                    r = 32 * sub
                    bcp = psf.tile([128, N], F32, tag="f", name="bcp")
                    nc.tensor.matmul(bcp[0:64, :], ones64[r:r + 1, :],
                                     rcb[r:r + 1, :], tile_position=(r, 0))
                    nc.vector.tensor_mul(
                        out=oT[64 * sub:64 * (sub + 1), hp, :],
                        in0=avs[64 * sub:64 * (sub + 1), :],
                        in1=bcp[0:64, :])

            emit_sc(*slots[0])
            for i, (hp, c) in enumerate(slots):
                if i + 1 < len(slots):
                    emit_sc(*slots[i + 1])
                feed_step()
                emit_av(hp, c)
                if c == NT - 1:
                    finish_pair(hp)
                if c == 2 and hp > 0:
                    emit_bcast(hp - 1)
            emit_bcast(CK - 1)

            # ---- proj (+2x via pre-scaled weights) ----
            for tt in range(NT):
                pw = psf.tile([128, 512], F32, tag="f", name="pw")
                ph = psf.tile([128, 512], F32, tag="f", name="ph")
                for k in range(CK):
                    nc.tensor.matmul(pw[:], oT[:, k, tt * 128:(tt + 1) * 128],
                                     wpT[:, k, 0:512],
                                     start=(k == 0), stop=(k == CK - 1))
                    nc.tensor.matmul(ph[:, 0:256],
                                     oT[:, k, tt * 128:(tt + 1) * 128],
                                     wpT[:, k, 512:768],
                                     start=(k == 0), stop=(k == CK - 1))
                feed_step()
                g = b * NT + tt
                nc.vector.tensor_add(out=x2r[:, g, 0:512], in0=pw[:],
                                     in1=pb2_bc[:, 0:512])
                nc.vector.tensor_add(out=x2r[:, g, 512:768], in0=ph[:, 0:256],
                                     in1=pb2_bc[:, 512:768])
            while fi[0] < len(feed):
                feed_step()
            if b + 1 < BPC:
                cur = nxt

    # ================= stage B: MLP =================
    with tc.tile_pool(name="wf2p", bufs=1) as wf2p, \
         tc.tile_pool(name="p2", bufs=1) as p2:
        wf2T = wf2p.tile([128, JH, C], BF16)
        load_wT(io["fc2_wT"], C, HID, wf2T)

        def ln2_chunk(ch):
            return [layer_norm(x2r[:, ch * NT + tt, :], p2)
                    for tt in range(NT)]

        h2T_cur, f1_0p = h2T0, f1_0
        for ch in range(G // NT):
            # ---- fc1 + gelu ----
            f1a = (f1_0p if ch == 0 else
                   h2p.tile([128, PREF, N], BF16, tag="f1a", bufs=1,
                            name="f1a"))
            f1b = p2.tile([128, JH - PREF, N], BF16, tag="f1b", bufs=1,
                          name="f1b")

            def f1_at(j):
                return f1a[:, j, :] if j < PREF else f1b[:, j - PREF, :]

            for j in range(PREF if ch == 0 else 0, JH):
                fp = psw.tile([128, N], F32, tag="w", name="fp")
                for k in range(CK):
                    nc.tensor.matmul(fp[:], wf1T[:, k, j * 128:(j + 1) * 128],
                                     h2T_cur[:, k, :], start=(k == 0),
                                     stop=(k == CK - 1))
                nc.scalar.activation(out=f1_at(j), in_=fp[:], func=AF.Gelu,
                                     bias=fc1b_t[:, j:j + 1])

            # LN2 of next chunk on DVE while fc1 runs
            if ch + 1 < G // NT:
                xns_n = ln2_chunk(ch + 1)
                h2T_next = h2p.tile([128, CK, N], BF16, tag="h2T", bufs=2,
                                    name="h2Tn")

            # ---- fc2 + residual, interleaved with next chunk transposes ----
            for tt in range(NT):
                g = ch * NT + tt
                x2pb = p2.tile([128, C], F32, tag="x2pb", bufs=2, name="x2pb")
                nc.vector.tensor_add(out=x2pb, in0=x2r[:, g, :], in1=fc2b_bc)
                f2a = psf.tile([128, 512], F32, tag="f", name="f2a")
                f2b = psf.tile([128, 512], F32, tag="f", name="f2b")
                for k in range(JH):
                    lhs = f1_at(k)[:, tt * 128:(tt + 1) * 128]
                    nc.tensor.matmul(f2a[:], lhs, wf2T[:, k, 0:512],
                                     start=(k == 0), stop=(k == JH - 1))
                    nc.tensor.matmul(f2b[:, 0:256], lhs, wf2T[:, k, 512:768],
                                     start=(k == 0), stop=(k == JH - 1))
                if ch + 1 < G // NT:
                    transpose_block(xns_n[tt], h2T_next, tt, ln2w_k, ln2b_k)
                o_t = p2.tile([128, C], F32, tag="outt", bufs=3, name="o_t")
                nc.vector.tensor_add(out=o_t[:, 0:512], in0=f2a[:],
                                     in1=x2pb[:, 0:512])
                nc.vector.tensor_add(out=o_t[:, 512:768], in0=f2b[:, 0:256],
                                     in1=x2pb[:, 512:768])
                nc.scalar.dma_start(
                    out=io["out"][g * 128:(g + 1) * 128, :], in_=o_t)
            if ch + 1 < G // NT:
                h2T_cur = h2T_next


_CACHE = {}


def _build():
    if "nc" in _CACHE:
        return _CACHE["nc"]
    nc = bacc.Bacc("TRN2", target_bir_lowering=False, debug=False,
                   num_devices=NCORES)
    io = {}
    io["x"] = nc.dram_tensor("x", [T, C], BF16, kind="ExternalInput").ap()
    for name, shape in [("qkv_wT", [C, 3 * C]), ("proj_wT", [C, C]),
                        ("fc1_wT", [C, HID]), ("fc2_wT", [HID, C])]:
        io[name] = nc.dram_tensor(name, shape, BF16, kind="ExternalInput").ap()
    for name, shape in [("ln1_w", [C]), ("ln1_b", [C]), ("ln2_w", [C]),
                        ("ln2_b", [C]), ("pb2", [C]), ("fc1_b", [HID]),
                        ("fc2_b", [C])]:
        io[name] = nc.dram_tensor(name, shape, F32, kind="ExternalInput").ap()
    io["out"] = nc.dram_tensor("out", [T, C], F32, kind="ExternalOutput").ap()

    with tile.TileContext(nc) as tc:
        with ExitStack() as ctx:
            _emit(tc, io, ctx)
    nc.compile()
    _CACHE["nc"] = nc
    return nc


def _in_maps(inputs):
    f = {k: np.ascontiguousarray(np.asarray(v, dtype=np.float32))
         for k, v in inputs.items()}
    x = f["x"].reshape(B, N, C)
    base = {
        "qkv_wT": np.ascontiguousarray(f["qkv_w"].T).astype(BF),
        "proj_wT": np.ascontiguousarray(2.0 * f["proj_w"].T).astype(BF),
        "fc1_wT": np.ascontiguousarray(f["fc1_w"].T).astype(BF),
        "fc2_wT": np.ascontiguousarray(f["fc2_w"].T).astype(BF),
        "ln1_w": f["ln1_w"], "ln1_b": f["ln1_b"],
        "ln2_w": f["ln2_w"], "ln2_b": f["ln2_b"],
        "pb2": 2.0 * f["proj_b"], "fc1_b": f["fc1_b"], "fc2_b": f["fc2_b"],
    }
    in_maps = []
    for c in range(NCORES):
        m = dict(base)
        m["x"] = np.ascontiguousarray(
            x[c * BPC:(c + 1) * BPC].reshape(T, C).astype(BF))
        in_maps.append(m)
    return in_maps


def kernel(**inputs):
    nc = _build()
    in_maps = _in_maps(inputs)
    res = run_bass_kernel_spmd(nc, in_maps, core_ids=list(range(NCORES)))
    out = np.concatenate(
        [r["out"].reshape(BPC, N, C) for r in res.results], axis=0)
    return out.astype(np.float32)


if __name__ == "__main__":
    rng = np.random.default_rng(0)
    ins = {
        "x": rng.standard_normal((B, N, C), dtype=np.float32),
        "ln1_w": np.ones(C, np.float32), "ln1_b": np.zeros(C, np.float32),
        "qkv_w": rng.standard_normal((3 * C, C), dtype=np.float32) / np.sqrt(C),
        "proj_w": rng.standard_normal((C, C), dtype=np.float32) / np.sqrt(C),
        "proj_b": np.zeros(C, np.float32),
        "ln2_w": np.ones(C, np.float32), "ln2_b": np.zeros(C, np.float32),
        "fc1_w": rng.standard_normal((HID, C), dtype=np.float32) / np.sqrt(C),
        "fc1_b": np.zeros(HID, np.float32),
        "fc2_w": rng.standard_normal((C, HID), dtype=np.float32) / np.sqrt(HID),
        "fc2_b": np.zeros(C, np.float32),
    }
    out = kernel(**ins)

    def ln(x, w, b):
        mu = x.mean(-1, keepdims=True)
        va = x.var(-1, keepdims=True)
        return (x - mu) / np.sqrt(va + EPS) * w + b

    x = ins["x"]
    h = ln(x, ins["ln1_w"], ins["ln1_b"])
    qkv = np.einsum('bnc,fc->bnf', h, ins["qkv_w"])
    qkv = qkv.reshape(B, N, 3, H, D).transpose(2, 0, 3, 1, 4)
    q, k, v = qkv[0], qkv[1], qkv[2]
    att = np.einsum('bhnd,bhmd->bhnm', q, k) * SCALE
    att = np.exp(att - att.max(-1, keepdims=True))
    att = att / att.sum(-1, keepdims=True)
    o = np.einsum('bhnm,bhmd->bhnd', att, v)
    o = o.transpose(0, 2, 1, 3).reshape(B, N, C)
    o = np.einsum('bnc,oc->bno', o, ins["proj_w"]) + ins["proj_b"]
    x2 = 2 * o
    h2 = ln(x2, ins["ln2_w"], ins["ln2_b"])
    h2 = np.einsum('bnc,hc->bnh', h2, ins["fc1_w"]) + ins["fc1_b"]
    from scipy.special import erf
    h2 = h2 * 0.5 * (1 + erf(h2 / np.sqrt(2)))
    h2 = np.einsum('bnh,oh->bno', h2, ins["fc2_w"]) + ins["fc2_b"]
    ref = x2 + h2
    err = np.abs(out - ref)
    print("out", out.shape, "absmax", np.abs(ref).max(),
          "maxerr", err.max(), "rel", err.max() / np.abs(ref).max())
